# revision 1
# baseline (speedup 1.0000x reference)
"""Single-head causal attention with softmax over the QUERY axis (dim=1).

out[b,i,d] = sum_j softmax_i(mask(q@kT/8))[i,j] * v[j,d]

Strategy: data-parallel over batch B=8, one batch element per NeuronCore.
Per core:
  - transpose x[b] (PE transposes) -> xT [C=384, S=2048]
  - qT = (Wq/8).T @ xT, kT = Wk.T @ xT   (both [64, 2048], d on partitions)
  - v  = x @ Wv                          ([2048, 64] as 16 [128,64] tiles)
  - for each key tile jt: scoresT[j, i] = kT_jt.T @ qT  (j on partitions,
    i on free axis) => softmax over i is a FREE-AXIS reduction, fused into
    the Exp activation via accum_out.  Causal mask handled additively on
    the diagonal block only (i >= j valid).
  - fold 1/denom_j into v rows: vs[j,:] = v[j,:] / denom[j], then
    out[i,:] += attnT_jt[:, i].T @ vs  accumulated in PSUM across jt.
"""

import numpy as np
import sys

sys.path.insert(0, "/opt/trn_rl_repo")

import concourse.bass as bass
import concourse.mybir as mybir
from concourse.bacc import Bacc
from concourse.tile import TileContext
from concourse.bass_utils import run_bass_kernel_spmd

B, S, C, D = 8, 2048, 384, 64
P = 128
NT = S // P  # 16 query/key tiles
CC = C // P  # 3 contraction chunks
F32 = mybir.dt.float32
F32R = mybir.dt.float32r
AFT = mybir.ActivationFunctionType
AX = mybir.AxisListType

_COMPILED = None
BUFS = {"ps": 6, "attnp": 3, "small": 6, "xsp": 4}


def build_nc():
    nc = Bacc()
    x_b = nc.declare_dram_parameter("x_b", [S, C], F32R, isOutput=False)
    wq = nc.declare_dram_parameter("wq", [C, D], F32R, isOutput=False)  # pre-scaled 1/8
    wk = nc.declare_dram_parameter("wk", [C, D], F32R, isOutput=False)
    wv = nc.declare_dram_parameter("wv", [C, D], F32, isOutput=False)
    ident = nc.declare_dram_parameter("ident", [P, P], F32R, isOutput=False)
    negmask = nc.declare_dram_parameter("negmask", [P, P], F32, isOutput=False)
    out_b = nc.declare_dram_parameter("out_b", [S, D], F32, isOutput=True)

    with TileContext(nc) as tc:
        with (
            tc.tile_pool(name="consts", bufs=1) as consts,
            tc.tile_pool(name="big", bufs=1) as big,
            tc.tile_pool(name="xsp", bufs=BUFS["xsp"]) as xsp,
            tc.tile_pool(name="attnp", bufs=BUFS["attnp"]) as attnp,
            tc.tile_pool(name="small", bufs=BUFS["small"]) as small,
            tc.tile_pool(name="psO", bufs=1, space="PSUM") as psO,
            tc.tile_pool(name="ps", bufs=BUFS["ps"], space="PSUM") as ps,
        ):
            # ---- constants ----
            idt = consts.tile([P, P], F32R)
            nc.sync.dma_start(out=idt, in_=ident[:, :])
            msk = consts.tile([P, P], F32)
            nc.sync.dma_start(out=msk, in_=negmask[:, :])
            wq_t = consts.tile([P, CC * D], F32R)
            wk_t = consts.tile([P, CC * D], F32R)
            wv_t = consts.tile([P, CC * D], F32)
            for wt, wd in ((wq_t, wq), (wk_t, wk), (wv_t, wv)):
                nc.sync.dma_start(
                    out=wt.rearrange("p (c d) -> p c d", c=CC),
                    in_=wd.ap().rearrange("(c p) d -> p c d", p=P),
                )

            # ---- persistent SBUF tensors ----
            xT = big.tile([P, CC * S], F32R)        # [128, 3*2048] xT chunks
            qk = big.tile([64, 2 * S], F32R)        # qT(scaled) | kT
            v_all = big.tile([P, NT * D], F32)     # v tiles [128, 16*64]
            out_sb = big.tile([P, NT * D], F32)    # final out staging

            # ---- phase A: load + transpose x (4 s-tiles per DMA) ----
            for g in range(NT // 4):
                xs = xsp.tile([P, 4 * C], F32R, tag="xs")
                nc.sync.dma_start(
                    out=xs.rearrange("p (t c) -> p t c", t=4),
                    in_=x_b[g * 4 * P:(g + 1) * 4 * P, :].rearrange(
                        "(t p) c -> p t c", p=P),
                )
                for c in range(CC):
                    pt4 = ps.tile([P, 512], F32, tag="ps")
                    for t in range(4):
                        nc.tensor.matmul(
                            pt4[:, t * P:(t + 1) * P].bitcast(F32R),
                            xs[:, t * C + c * P: t * C + (c + 1) * P], idt,
                            is_transpose=True, start=(t == 0), stop=(t == 3),
                        )
                    nc.vector.tensor_copy(
                        xT[:, c * S + g * 4 * P: c * S + (g + 1) * 4 * P], pt4
                    )

            # ---- qT / kT: [64, 2048] = W.T @ xT ----
            for n in range(S // 512):
                pq = ps.tile([64, 512], F32, tag="ps")
                for c in range(CC):
                    nc.tensor.matmul(
                        pq, wq_t[:, c * D:(c + 1) * D],
                        xT[:, c * S + n * 512: c * S + (n + 1) * 512],
                        start=(c == 0), stop=(c == CC - 1),
                    )
                nc.vector.tensor_copy(qk[:, n * 512:(n + 1) * 512], pq)
                pk = ps.tile([64, 512], F32, tag="ps")
                for c in range(CC):
                    nc.tensor.matmul(
                        pk, wk_t[:, c * D:(c + 1) * D],
                        xT[:, c * S + n * 512: c * S + (n + 1) * 512],
                        start=(c == 0), stop=(c == CC - 1),
                    )
                nc.vector.tensor_copy(qk[:, S + n * 512: S + (n + 1) * 512], pk)

            # ---- v tiles [128, 64] = xT_chunk.T @ Wv ----
            for st in range(NT):
                pv = ps.tile([P, D], F32, tag="ps")
                for c in range(CC):
                    nc.tensor.matmul(
                        pv, xT[:, c * S + st * P: c * S + (st + 1) * P].bitcast(F32),
                        wv_t[:, c * D:(c + 1) * D],
                        start=(c == 0), stop=(c == CC - 1),
                    )
                nc.vector.tensor_copy(v_all[:, st * D:(st + 1) * D], pv)

            # ---- phase B: per key-tile softmax + accumulation ----
            # Software-pipelined: scores+exp for jt+1 are emitted BEFORE the
            # softmax tail + attn@v matmuls of jt, so PE works on scores_{jt+1}
            # while ACT/DVE finish the softmax chain of jt.
            outp = psO.tile([P, NT * D], F32)  # [128, 1024] accumulator, 2 banks

            def emit_scores(jt):
                Ni = S - jt * P  # valid queries i >= jt*128
                atile = attnp.tile([P, S], F32, tag="attn", name=f"atile{jt}")
                dens = small.tile([P, 4], F32, tag="dens", name=f"dens{jt}")
                nch = (Ni + 511) // 512
                for ci in range(nch):
                    w = min(512, Ni - ci * 512)
                    i0 = jt * P + ci * 512
                    sc = ps.tile([P, 512], F32, tag="ps", name=f"sc{jt}_{ci}")
                    nc.tensor.matmul(
                        sc[:, :w],
                        qk[:, S + jt * P: S + (jt + 1) * P],
                        qk[:, i0: i0 + w],
                        start=True, stop=True,
                    )
                    if ci == 0:
                        # causal mask on diagonal block: -1e30 where i < j
                        nc.vector.tensor_add(sc[:, :P], sc[:, :P], msk)
                    nc.scalar.activation(
                        atile[:, ci * 512: ci * 512 + w], sc[:, :w], AFT.Exp,
                        accum_out=dens[:, ci: ci + 1],
                    )
                return atile, dens, nch

            pend = emit_scores(0)
            for jt in range(NT):
                atile, dens, nch = pend
                if jt + 1 < NT:
                    pend = emit_scores(jt + 1)
                if nch == 1:
                    den = dens[:, 0:1]  # single chunk: accum_out IS the row sum
                else:
                    den_t = small.tile([P, 1], F32, tag="den")
                    nc.vector.reduce_sum(den_t, dens[:, :nch], axis=AX.X)
                    den = den_t
                rv = small.tile([P, 1], F32, tag="rv")
                nc.vector.reciprocal(rv, den)
                vs = small.tile([P, D], F32, tag="vs")
                nc.vector.tensor_scalar_mul(vs, v_all[:, jt * D:(jt + 1) * D], rv)
                for it in range(jt, NT):
                    # outp is 2 PSUM banks (it 0..7 | 8..15). start=True zeroes
                    # the whole 2KB bank, so only the first matmul touching each
                    # bank starts; the last touching each bank stops.
                    bank_first = jt == 0 and it in (0, 8)
                    bank_last = (jt == 7 and it == 7) or (jt == 15 and it == 15)
                    nc.tensor.matmul(
                        outp[:, it * D:(it + 1) * D],
                        atile[:, (it - jt) * P:(it - jt + 1) * P],  # [128j,128i]
                        vs,
                        start=bank_first, stop=bank_last,
                    )

            nc.vector.tensor_copy(out_sb, outp)
            nc.sync.dma_start(
                out=out_b.ap().rearrange("(t p) d -> p t d", p=P),
                in_=out_sb.rearrange("p (t d) -> p t d", t=NT),
            )
    nc.finalize()
    return nc


def _build_inputs(x, Wq, Wk, Wv):
    x = np.ascontiguousarray(np.asarray(x, dtype=np.float32))
    wq_s = np.ascontiguousarray(np.asarray(Wq, dtype=np.float32) * np.float32(D ** -0.5))
    wk_ = np.ascontiguousarray(np.asarray(Wk, dtype=np.float32))
    wv_ = np.ascontiguousarray(np.asarray(Wv, dtype=np.float32))
    ident = np.eye(P, dtype=np.float32)
    r = np.arange(P)
    negmask = np.where(r[None, :] >= r[:, None], 0.0, -1e30).astype(np.float32)
    return [
        {"x_b": x[b], "wq": wq_s, "wk": wk_, "wv": wv_,
         "ident": ident, "negmask": negmask}
        for b in range(B)
    ]


def kernel(x, Wq, Wk, Wv, _trace=False):
    global _COMPILED
    if _COMPILED is None:
        _COMPILED = build_nc()
    nc = _COMPILED
    in_maps = _build_inputs(x, Wq, Wk, Wv)
    res = run_bass_kernel_spmd(nc, in_maps, core_ids=list(range(B)), trace=_trace)
    out = np.stack([res.results[b]["out_b"] for b in range(B)], axis=0).astype(np.float32)
    if _trace:
        kernel.last_results = res
    return out



# revision 6
# speedup vs baseline: 1.3365x; 1.3365x over previous
"""Single-head causal attention with softmax over the QUERY axis (dim=1).

out[b,i,d] = sum_j softmax_i(mask(q@kT/8))[i,j] * v[j,d]

Data-parallel over batch B=8, one batch element per NeuronCore.

Per-core pipeline (all matmuls bf16 at 1 cycle/row; fp32r only for the
x transposes whose input dtype is fixed by the DMA):
  - x arrives in 4 s-groups of 512 rows (group 3 first, split 128+384 so
    the PE can start transposing ~1.2us earlier).
  - per group g (descending): PE-transpose x -> xT (bf16), project
    qT/kT with a packed [Wq*8^-1 | Wk] weight (one 128-wide matmul chain
    instead of two 64-wide ones), project v tiles.
  - key tiles jt processed in DESCENDING order so tile 15 only needs
    group 3 of phase A: scoresT[j,i] = kT_jt.T @ qT (j on partitions,
    i free) in 1024-col PSUM chunks; causal mask added via a PE matmul
    (identity @ negmask accumulated into the diagonal bank); one Exp
    activation per chunk with accum_out giving the softmax denominator
    for free; vs[j,:] = v[j,:]/den[j] on GPSIMD; out[i,:] += attnT.T@vs
    accumulated across jt in a persistent 2-bank PSUM accumulator.
  - output DMA'd in 2 halves so the first half overlaps the last avs.
"""

import numpy as np
import sys

sys.path.insert(0, "/opt/trn_rl_repo")

import ml_dtypes
import concourse.bass as bass
import concourse.mybir as mybir
from concourse.bacc import Bacc
from concourse.tile import TileContext
from concourse.bass_utils import run_bass_kernel_spmd

B, S, C, D = 8, 2048, 384, 64
P = 128
NT = S // P   # 16 s-tiles
CC = C // P   # 3 contraction chunks
G = 4         # s-groups of 4 tiles (512 rows)
F32 = mybir.dt.float32
F32R = mybir.dt.float32r
BF16 = mybir.dt.bfloat16
AFT = mybir.ActivationFunctionType
AX = mybir.AxisListType

_COMPILED = None


def build_nc():
    nc = Bacc()
    x_b = nc.declare_dram_parameter("x_b", [S, C], F32R, isOutput=False)
    wqk = nc.declare_dram_parameter("wqk", [C, P], BF16, isOutput=False)  # [Wq/8 | Wk]
    wv = nc.declare_dram_parameter("wv", [C, D], BF16, isOutput=False)
    ident = nc.declare_dram_parameter("ident", [P, P], F32R, isOutput=False)
    identb = nc.declare_dram_parameter("identb", [P, P], BF16, isOutput=False)
    negmask = nc.declare_dram_parameter("negmask", [P, P], BF16, isOutput=False)
    out_b = nc.declare_dram_parameter("out_b", [S, D], F32, isOutput=True)

    with TileContext(nc) as tc:
        with (
            tc.tile_pool(name="consts", bufs=1) as consts,
            tc.tile_pool(name="big", bufs=1) as big,
            tc.tile_pool(name="xsp", bufs=2) as xsp,
            tc.tile_pool(name="attnp", bufs=3) as attnp,
            tc.tile_pool(name="small", bufs=8) as small,
            tc.tile_pool(name="vsp", bufs=3) as vsp,
            tc.tile_pool(name="psO", bufs=1, space="PSUM") as psO,
            tc.tile_pool(name="psS", bufs=2, space="PSUM") as psS,
            tc.tile_pool(name="psA", bufs=2, space="PSUM") as psA,
        ):
            # ---- constants ----
            idt = consts.tile([P, P], F32R)
            idb = consts.tile([P, P], BF16)
            msk = consts.tile([P, P], BF16)
            wqk_t = consts.tile([P, CC * P], BF16)
            wv_t = consts.tile([P, CC * D], BF16)
            trash = consts.tile([1, 2], F32)

            # preload the Exp table while DMAs are still in flight
            nc.vector.memset(trash, 0.0)
            nc.scalar.activation(trash[:, 0:1], trash[:, 1:2], AFT.Exp)

            nc.sync.dma_start(out=idt, in_=ident[:, :])
            nc.sync.dma_start(
                out=wqk_t.rearrange("p (c d) -> p c d", c=CC),
                in_=wqk.ap().rearrange("(c p) d -> p c d", p=P),
            )

            # x staging: group 3 split (tile 15 alone) so transposes start early
            xs_h = xsp.tile([P, C], F32R, tag="xsh", bufs=1)  # s-tile 15
            xs_r = xsp.tile([P, 3 * C], F32R, tag="xs", bufs=3)  # s-tiles 12..14
            nc.sync.dma_start(out=xs_h, in_=x_b[15 * P:16 * P, :])
            nc.sync.dma_start(
                out=xs_r.rearrange("p (t c) -> p t c", t=3),
                in_=x_b[12 * P:15 * P, :].rearrange("(t p) c -> p t c", p=P),
            )
            nc.sync.dma_start(
                out=wv_t.rearrange("p (c d) -> p c d", c=CC),
                in_=wv.ap().rearrange("(c p) d -> p c d", p=P),
            )
            nc.sync.dma_start(out=idb, in_=identb[:, :])
            nc.sync.dma_start(out=msk, in_=negmask[:, :])
            xs_g = {}
            for g in (2, 1, 0):
                xs_g[g] = xsp.tile([P, 4 * C], F32R, tag="xs", bufs=3,
                                   name=f"xs{g}")
                nc.sync.dma_start(
                    out=xs_g[g].rearrange("p (t c) -> p t c", t=4),
                    in_=x_b[g * 4 * P:(g + 1) * 4 * P, :].rearrange(
                        "(t p) c -> p t c", p=P),
                )

            # ---- persistent SBUF tensors ----
            xT = big.tile([P, CC * S], BF16)       # [128, 3*2048] xT (bf16)
            qt_sb = big.tile([64, S], BF16)        # qT [d, s], pre-scaled 1/8
            kt_sb = big.tile([64, S], BF16)        # kT [d, s]
            v_all = big.tile([P, NT * D], BF16)    # v tiles [128, 16*64]
            out_sb = big.tile([P, NT * D], F32)    # final out staging

            outp = psO.tile([P, NT * D], F32)      # [128, 1024] accumulator

            def emit_A(g):
                """transpose 4 s-tiles of group g, project qT/kT and v."""
                # transposes: per c-chunk, 4 [128,128] transposes into 1 bank
                ts = [3, 0, 1, 2] if g == 3 else [0, 1, 2, 3]
                for c in range(CC):
                    pt = psA.tile([P, 512], F32, tag="psA", name=f"pt{g}_{c}")
                    for k, t in enumerate(ts):
                        if g == 3 and t == 3:
                            src = xs_h[:, c * P:(c + 1) * P]
                        elif g == 3:
                            src = xs_r[:, t * C + c * P: t * C + (c + 1) * P]
                        else:
                            src = xs_g[g][:, t * C + c * P: t * C + (c + 1) * P]
                        nc.tensor.matmul(
                            pt[:, t * P:(t + 1) * P].bitcast(F32R), src, idt,
                            is_transpose=True, start=(k == 0), stop=(k == 3),
                        )
                    nc.vector.tensor_copy(
                        xT[:, c * S + g * 512: c * S + (g + 1) * 512], pt)
                # qT / kT projections: [64, 512] psum each, partition base 0
                pq = psA.tile([64, 512], F32, tag="psA", name=f"pq{g}")
                for c in range(CC):
                    nc.tensor.matmul(
                        pq, wqk_t[:, c * P: c * P + 64],
                        xT[:, c * S + g * 512: c * S + (g + 1) * 512],
                        start=(c == 0), stop=(c == CC - 1),
                    )
                nc.vector.tensor_copy(qt_sb[:, g * 512:(g + 1) * 512], pq)
                pk = psA.tile([64, 512], F32, tag="psA", name=f"pk{g}")
                for c in range(CC):
                    nc.tensor.matmul(
                        pk, wqk_t[:, c * P + 64: c * P + 128],
                        xT[:, c * S + g * 512: c * S + (g + 1) * 512],
                        start=(c == 0), stop=(c == CC - 1),
                    )
                nc.vector.tensor_copy(kt_sb[:, g * 512:(g + 1) * 512], pk)
                # v tiles [128, 64]
                for t in range(4):
                    st = g * 4 + t
                    pv = psA.tile([P, 512], F32, tag="psA", name=f"pv{st}")
                    for c in range(CC):
                        nc.tensor.matmul(
                            pv[:, 0:D],
                            xT[:, c * S + st * P: c * S + (st + 1) * P],
                            wv_t[:, c * D:(c + 1) * D],
                            start=(c == 0), stop=(c == CC - 1),
                        )
                    nc.vector.tensor_copy(v_all[:, st * D:(st + 1) * D], pv[:, 0:D])

            def emit_scores(jt):
                """scoresT chunks + mask + Exp; returns (atile, dens, nch)."""
                Ni = S - jt * P
                atile = attnp.tile([P, S], BF16, tag="attn", name=f"atile{jt}")
                dens = small.tile([P, 2], F32, tag="dens", name=f"dens{jt}")
                nch = (Ni + 1023) // 1024
                for ci in range(nch):
                    w = min(1024, Ni - ci * 1024)
                    i0 = jt * P + ci * 1024
                    sc = psS.tile([P, 1024], F32, tag="psS", name=f"sc{jt}_{ci}")
                    for sub in range((w + 511) // 512):
                        sw = min(512, w - sub * 512)
                        diag = ci == 0 and sub == 0
                        nc.tensor.matmul(
                            sc[:, sub * 512: sub * 512 + sw],
                            kt_sb[:, jt * P:(jt + 1) * P],
                            qt_sb[:, i0 + sub * 512: i0 + sub * 512 + sw],
                            start=True, stop=not diag,
                        )
                    if ci == 0:
                        # causal mask on the diagonal block: += -1e30 (i < j)
                        nc.tensor.matmul(
                            sc[:, 0:P], idb, msk, start=False, stop=True)
                    nc.scalar.activation(
                        atile[:, ci * 1024: ci * 1024 + w], sc[:, :w], AFT.Exp,
                        accum_out=dens[:, ci: ci + 1],
                    )
                return atile, dens, nch

            def emit_tail(jt, atile, dens, nch):
                if nch == 1:
                    den = dens[:, 0:1]
                else:
                    den_t = small.tile([P, 1], F32, tag="den", name=f"den{jt}")
                    nc.vector.reduce_sum(den_t, dens[:, :nch], axis=AX.X)
                    den = den_t
                rv = small.tile([P, 1], F32, tag="rv", name=f"rv{jt}")
                nc.vector.reciprocal(rv, den)
                vs = vsp.tile([P, D], BF16, tag="vs", name=f"vs{jt}")
                nc.gpsimd.tensor_scalar_mul(vs, v_all[:, jt * D:(jt + 1) * D], rv)
                its = list(range(jt, NT)) if jt > 0 else list(range(15, 7, -1))
                for it in its:
                    nc.tensor.matmul(
                        outp[:, it * D:(it + 1) * D],
                        atile[:, (it - jt) * P:(it - jt + 1) * P], vs,
                        start=(jt == 15 and it == 15) or (jt == 7 and it == 7),
                        stop=(jt == 0 and it == 8),
                    )
                if jt == 0:
                    # bank2 (s 1024..2047) is final: drain it while bank1 avs run
                    nc.vector.tensor_copy(out_sb[:, 512:1024], outp[:, 512:1024])
                    nc.sync.dma_start(
                        out=out_b[8 * P:16 * P, :].rearrange(
                            "(t p) d -> p t d", p=P),
                        in_=out_sb[:, 512:1024].rearrange("p (t d) -> p t d", t=8),
                    )
                    for it in range(7, -1, -1):
                        nc.tensor.matmul(
                            outp[:, it * D:(it + 1) * D],
                            atile[:, it * P:(it + 1) * P], vs,
                            start=False, stop=(it == 0),
                        )
                    nc.vector.tensor_copy(out_sb[:, 0:512], outp[:, 0:512])
                    nc.sync.dma_start(
                        out=out_b[0:8 * P, :].rearrange("(t p) d -> p t d", p=P),
                        in_=out_sb[:, 0:512].rearrange("p (t d) -> p t d", t=8),
                    )

            # ---- software-pipelined main loop, jt descending ----
            pend = None
            for g in (3, 2, 1, 0):
                emit_A(g)
                for jt in range(g * 4 + 3, g * 4 - 1, -1):
                    if pend is None:
                        pend = (jt, *emit_scores(jt))
                        continue
                    pjt, atile, dens, nch = pend
                    pend = (jt, *emit_scores(jt))
                    emit_tail(pjt, atile, dens, nch)
            pjt, atile, dens, nch = pend
            emit_tail(pjt, atile, dens, nch)

    nc.finalize()
    return nc


def _bf16(a):
    return np.ascontiguousarray(np.asarray(a, dtype=np.float32)).astype(
        ml_dtypes.bfloat16)


def _build_inputs(x, Wq, Wk, Wv):
    x = np.ascontiguousarray(np.asarray(x, dtype=np.float32))
    wq_s = np.asarray(Wq, dtype=np.float32) * np.float32(D ** -0.5)
    wqk = _bf16(np.concatenate([wq_s, np.asarray(Wk, dtype=np.float32)], axis=1))
    wv_ = _bf16(Wv)
    ident = np.eye(P, dtype=np.float32)
    identb = _bf16(ident)
    r = np.arange(P)
    negmask = _bf16(np.where(r[None, :] >= r[:, None], 0.0, -1e30))
    return [
        {"x_b": x[b], "wqk": wqk, "wv": wv_, "ident": ident,
         "identb": identb, "negmask": negmask}
        for b in range(B)
    ]


def kernel(x, Wq, Wk, Wv, _trace=False):
    global _COMPILED
    if _COMPILED is None:
        _COMPILED = build_nc()
    nc = _COMPILED
    in_maps = _build_inputs(x, Wq, Wk, Wv)
    res = run_bass_kernel_spmd(nc, in_maps, core_ids=list(range(B)), trace=_trace)
    out = np.stack([res.results[b]["out_b"] for b in range(B)], axis=0).astype(np.float32)
    if _trace:
        kernel.last_results = res
    return out
